# revision 24
# baseline (speedup 1.0000x reference)
"""ChannelAttention Trainium2 kernel.

Reference computation (per batch b, group o):
    p_mean[s, c] = mean over (h, w) of x[b, o, s, c, :, :]
    p_max[s, c]  = max  over (h, w) of x[b, o, s, c, :, :]
    out = sigmoid(relu(p_mean @ w1[o].T) @ w2[o].T + relu(p_max @ w1[o].T) @ w2[o].T)
    result[b, o, s, c, 0, 0] = out[s, c]

Strategy: data-parallel over batch B=8 -> one batch per NeuronCore (64 MiB
of x per core; the kernel is HBM-bandwidth bound streaming x at the
~410 GB/s 16-DMA-engine per-core cap, so every compute engine must keep
up with the stream or the kernel grows a serial tail).

Per core, x[b] is viewed as [O*S*C, H*W] = [16384, 1024] and streamed in
2 MiB tiles of [128 partitions, 4, 1024] (4 row-blocks per chunk). The
reduction work per 1024-wide row-block is split to keep both the DVE and
the Act engine below the 5.12 us/chunk stream cadence:

  - blocks 0..2 of each chunk ("scalar blocks"): the Act engine runs a
    Copy activation whose accumulator produces the fp32 row sum and whose
    main output is a bf16 copy of the block (the copy is otherwise junk,
    so the dtype conversion is free). The DVE then computes the block max
    from the bf16 copy with two 2x-mode tensor_tensor max-folds
    (bf16 elementwise ops run 2 elem/lane/cycle) plus one 1x
    tensor_reduce over the 4x-folded data -- ~0.72 us/block instead of
    the 1.2 us a direct fp32 tensor_reduce costs.
  - block 3 ("DVE block"): DVE does the max directly (fp32 tensor_reduce,
    bf16 output) and the sum with one dual-read scalar_tensor_tensor
    fold-add whose accum_out emits the full row sum (~0.7 us total vs
    1.35 us on the Act engine).

Pooled results land as [partition = (s%2)*64 + c, column = o*16 + s//2];
pooled_max is bf16 (well within the 2e-2 tolerance), pooled_sum fp32.
The tiny grouped MLP consumes that layout via block-diagonal duplicated
weights built host-side: the mean-path FC1 weight stays fp32 (with the
1/1024 mean scale folded in); the max-path FC1 and the shared FC2 run as
bf16 matmuls (hidden activations are written bf16 by the relu). Weight
loads and the small output stores go through the GpSimd SWDGE ring so
they never head-of-line block the x stream on the sync HWDGE ring.
"""

import numpy as np
import ml_dtypes

import concourse.bacc as bacc
import concourse.bass as bass
import concourse.mybir as mybir
import concourse.tile as tile
from concourse.bass_utils import run_bass_kernel_spmd

B, O, S, C, H, W = 8, 8, 32, 64, 32, 32
HID = C
HWSZ = H * W            # 1024 elements pooled per (b, o, s, c)
ROWS = O * S * C        # 16384 rows per core
RB = 128                # rows per partition block
T = ROWS // RB          # 128 row-blocks per core
JB = 4                  # row-blocks per stream tile (2 MiB DMAs)
SP = S // 2             # 16 pooled columns per group
N_CORES = 8

# 2 MiB chunks for the bulk of the stream, tapering at the end so the
# final reduce->MLP->store chain starts on less data (shorter tail). The
# taper blocks all run "p-style" (Act sum parallel to a DVE fp32 max) so
# neither engine exceeds the 1.28 us/block stream cadence at the end.
CHUNKS = [1, 1, 2] + [4] * 30 + [2, 1, 1]
assert sum(CHUNKS) == T

_CACHE = {}

fp32 = mybir.dt.float32
bf16 = mybir.dt.bfloat16


def _build_nc(chunks=None, jbmax=JB, style="v2"):
    if chunks is None:
        chunks = CHUNKS if style == "v5" else [4] * 31 + [2, 1, 1]
    assert sum(chunks) == T
    nc = bacc.Bacc(
        "TRN2", target_bir_lowering=False, debug=False, num_devices=N_CORES
    )
    x = nc.dram_tensor("x", [ROWS, HWSZ], fp32, kind="ExternalInput")
    wsum = nc.dram_tensor("wsum", [128, O * 128], fp32, kind="ExternalInput")
    wbf = nc.dram_tensor("wbf", [128, 2 * O * 128], bf16, kind="ExternalInput")
    out = nc.dram_tensor("out", [O * S, C], fp32, kind="ExternalOutput")

    AF = mybir.ActivationFunctionType
    ALU = mybir.AluOpType
    AX = mybir.AxisListType

    with tile.TileContext(nc) as tc:
        with (
            tc.tile_pool(name="xp", bufs=32 // jbmax) as xp,
            tc.tile_pool(name="bp", bufs=4) as bp,
            tc.tile_pool(name="small", bufs=1) as sp,
            tc.tile_pool(name="psum1", bufs=2, space=bass.MemorySpace.PSUM) as pp1,
            tc.tile_pool(name="psum2", bufs=2, space=bass.MemorySpace.PSUM) as pp2,
        ):
            wsum_sb = sp.tile([128, O * 128], fp32)
            wbf_sb = sp.tile([128, 2 * O * 128], bf16)

            pooled_sum = sp.tile([128, T], fp32)
            pooled_max = sp.tile([128, T], bf16)
            junk = sp.tile([128, 512], fp32)
            h_sb = sp.tile([128, O * 2 * SP], bf16)
            att = sp.tile([SP, O * 128], fp32)

            xv = x.ap().rearrange("(t p) f -> t p f", p=RB)
            ov = out.ap().rearrange("(o j r) c -> o j r c", o=O, j=SP, r=2)

            def mlp(o):
                w1s = wsum_sb[:, o * 128 : (o + 1) * 128]
                w1m = wbf_sb[:, o * 128 : (o + 1) * 128]
                w2b = wbf_sb[:, O * 128 + o * 128 : O * 128 + (o + 1) * 128]
                ps1m = pp1.tile([128, SP], fp32, tag="ps1m")
                ps1x = pp1.tile([128, SP], fp32, tag="ps1x")
                nc.tensor.matmul(ps1m[:], w1s, pooled_sum[:, o * SP : (o + 1) * SP])
                nc.tensor.matmul(ps1x[:], w1m, pooled_max[:, o * SP : (o + 1) * SP])
                hm = h_sb[:, o * 2 * SP : o * 2 * SP + SP]
                hx = h_sb[:, o * 2 * SP + SP : (o + 1) * 2 * SP]
                # relus on the DVE (tiny PSUM->SBUF tensor_scalar ops):
                # keeps the Act engine queue free of cross-engine matmul
                # waits so the next chunk's sum-activations start on time
                nc.vector.tensor_scalar_max(hm, ps1m[:], 0.0)
                nc.vector.tensor_scalar_max(hx, ps1x[:], 0.0)
                ps2 = pp2.tile([SP, 128], fp32, tag="ps2")
                nc.tensor.matmul(ps2[:], hm, w2b, start=True, stop=False)
                nc.tensor.matmul(ps2[:], hx, w2b, start=False, stop=True)
                ao = att[:, o * 128 : (o + 1) * 128]
                nc.scalar.activation(ao, ps2[:], AF.Sigmoid)
                # The last group's store rides the (by then idle) sync HWDGE
                # ring for lower latency; earlier stores use the GpSimd SWDGE
                # ring so they never head-of-line block x-chunk descriptors.
                eng = nc.sync if o == O - 1 else nc.gpsimd
                eng.dma_start(ov[o], ao.rearrange("p (r c) -> p r c", r=2))

            def chain(tc0, jb, nb, xt, xb):
                # Max-fold chain for all jb blocks of the chunk starting at
                # column tc0. s-blocks [0,nb) fold from their bf16 copies at
                # 2x-mode rate; blocks [nb,jb) fold straight from fp32 xt
                # (1x, but still half the cost of a direct fp32 reduce).
                # Runs one chunk AFTER its blocks streamed (the bf16 copies
                # finished a full slot earlier), so the DVE never stalls
                # waiting on the Act engine mid-stream.
                m1 = bp.tile([RB, jbmax, 512], bf16, tag="m1")
                m2 = bp.tile([RB, jbmax, 256], bf16, tag="m2")
                m3 = bp.tile([RB, jbmax, 128], bf16, tag="m3")
                nc.vector.tensor_tensor(
                    m1[:, :nb, :], xb[:, :nb, 0:512], xb[:, :nb, 512:1024], ALU.max
                )
                if jb > nb:
                    nc.vector.tensor_tensor(
                        m1[:, nb:jb, :],
                        xt[:, nb:jb, 0:512],
                        xt[:, nb:jb, 512:1024],
                        ALU.max,
                    )
                nc.vector.tensor_tensor(
                    m2[:, :jb, :], m1[:, :jb, 0:256], m1[:, :jb, 256:512], ALU.max
                )
                nc.vector.tensor_tensor(
                    m3[:, :jb, :], m2[:, :jb, 0:128], m2[:, :jb, 128:256], ALU.max
                )
                nc.vector.tensor_reduce(
                    pooled_max[:, tc0 : tc0 + jb],
                    m3[:, :jb, :],
                    axis=AX.X,
                    op=ALU.max,
                )

            pending_chain = None
            pending_mlps = []
            emitted = 0  # pooled columns fully emitted (contiguous prefix)
            t0 = 0
            for i, jb in enumerate(chunks):
                xt = xp.tile([RB, jbmax, HWSZ], fp32, tag="xt")
                nc.sync.dma_start(
                    xt[:, :jb, :], xv[t0 : t0 + jb].transpose([1, 0, 2])
                )
                if i == 2:
                    # Weight loads ride the sync HWDGE ring after the stream
                    # has ramped: they land ~22us, well before the first MLP
                    # needs them (~29us), without disturbing the ramp the way
                    # a concurrent GpSimd SWDGE load does.
                    nc.sync.dma_start(wsum_sb[:], wsum.ap())
                    nc.sync.dma_start(wbf_sb[:], wbf.ap())
                # Per-chunk block split, keeping both engines under the
                # 5.12 us/chunk stream cadence:
                #   s-blocks: Act engine sum (+ bf16 copy as the act output);
                #             max via the pipelined fold chain next slot.
                #   d-blocks: DVE fold-add sum; max joins the same chain.
                #   Taper: p-blocks (Act sum parallel to an immediate DVE
                #   fp32 max) and d-blocks with immediate fp32 maxes, so the
                #   kernel tail has no cross-engine chain latency.
                if jb >= 4:
                    s_blocks, d_blocks, p_blocks = (0, 1, 2), (3,), ()
                    chained = True
                elif style == "v5" and jb == 2 and t0 == T - 4:
                    # last 2-block chunk: sums split Act/DVE, maxes via the
                    # pipelined chain (it drains during the final singles)
                    s_blocks, d_blocks, p_blocks = (0,), (1,), ()
                    chained = True
                elif jb == 2:
                    s_blocks, p_blocks, d_blocks = (), (0,), (1,)
                    chained = False
                else:
                    s_blocks, p_blocks, d_blocks = (), (0,), ()
                    chained = False
                if pending_chain is not None:
                    chain(*pending_chain)
                    pending_chain = None
                xb = bp.tile([RB, jbmax, HWSZ], bf16, tag="xb")
                for j in (*s_blocks, *p_blocks):
                    nc.scalar.activation(
                        xb[:, j, :],
                        xt[:, j, :],
                        AF.Copy,
                        accum_out=pooled_sum[:, t0 + j : t0 + j + 1],
                    )
                for j in d_blocks:
                    nc.vector.scalar_tensor_tensor(
                        junk[:],
                        xt[:, j, 0:512],
                        0.0,
                        xt[:, j, 512:1024],
                        ALU.add,
                        ALU.add,
                        accum_out=pooled_sum[:, t0 + j : t0 + j + 1],
                    )
                if not chained:
                    for j in (*p_blocks, *d_blocks):
                        t = t0 + j
                        nc.vector.tensor_reduce(
                            pooled_max[:, t : t + 1],
                            xt[:, j, :],
                            axis=AX.X,
                            op=ALU.max,
                        )
                elif style == "v2":
                    # v2: immediate chain over all blocks -- the d-block's
                    # max enters via a half-cost fp32 fold (TT dual-read)
                    # instead of a full-width fp32 tensor_reduce
                    chain(t0, jb, len(s_blocks), xt, xb)
                else:
                    pending_chain = (t0, jb, len(s_blocks), xt, xb)
                # MLPs whose pooled inputs completed LAST slot: emitting
                # them here (after this chunk's acts) keeps their relu /
                # sigmoid from blocking the Act engine queue while the
                # cross-engine matmul chain resolves.
                if style == "v2":
                    pending_mlps = []
                for o in pending_mlps:
                    mlp(o)
                pending_mlps = []
                done = t0 + jb
                # Pooled columns fully emitted so far: everything before this
                # chunk, plus this chunk's own blocks unless their maxes wait
                # on the (still pending) fold chain.
                prefix = t0 if pending_chain is not None else done
                for o in range(O):
                    if emitted < (o + 1) * SP <= prefix:
                        if style == "v2":
                            mlp(o)
                        else:
                            pending_mlps.append(o)
                emitted = prefix
                t0 = done
            if pending_chain is not None:
                chain(*pending_chain)
            for o in pending_mlps:
                mlp(o)
            for o in range(O):
                if emitted < (o + 1) * SP <= T:
                    mlp(o)

    nc.compile()
    return nc


def _build_weights(w1, w2):
    """Block-diagonal duplicated weight images.

    wsum: fp32 [128, O*128] -- w1[o].T scaled by 1/HWSZ (mean path
          consumes raw row sums).
    wbf:  bf16 [128, 2*O*128] -- section 0: w1[o].T (max path),
          section 1: w2[o].T (shared FC2).
    """
    wsum = np.zeros((128, O * 128), dtype=np.float32)
    wbf = np.zeros((128, 2 * O * 128), dtype=ml_dtypes.bfloat16)
    for o in range(O):
        w1t = np.ascontiguousarray(w1[o].T)  # [C, HID]
        w2t = np.ascontiguousarray(w2[o].T)  # [HID, C]
        base = o * 128
        wsum[0:64, base : base + 64] = w1t / HWSZ
        wsum[64:128, base + 64 : base + 128] = w1t / HWSZ
        wbf[0:64, base : base + 64] = w1t.astype(ml_dtypes.bfloat16)
        wbf[64:128, base + 64 : base + 128] = w1t.astype(ml_dtypes.bfloat16)
        base = O * 128 + o * 128
        wbf[0:64, base : base + 64] = w2t.astype(ml_dtypes.bfloat16)
        wbf[64:128, base + 64 : base + 128] = w2t.astype(ml_dtypes.bfloat16)
    return wsum, wbf


def _build_in_maps(x, w1, w2):
    x = np.ascontiguousarray(x, dtype=np.float32).reshape(B, ROWS, HWSZ)
    wsum, wbf = _build_weights(
        np.asarray(w1, dtype=np.float32), np.asarray(w2, dtype=np.float32)
    )
    return [{"x": x[b], "wsum": wsum, "wbf": wbf} for b in range(B)]


def kernel(x, w1, w2):
    if "nc" not in _CACHE:
        _CACHE["nc"] = _build_nc()
    nc = _CACHE["nc"]
    in_maps = _build_in_maps(x, w1, w2)
    res = run_bass_kernel_spmd(nc, in_maps, core_ids=list(range(N_CORES)))
    out = np.stack([res.results[b]["out"] for b in range(B)])
    return out.reshape(B, O, S, C, 1, 1).astype(np.float32)
